# revision 9
# baseline (speedup 1.0000x reference)
"""Conditional BatchNorm1d (training mode) on 8 Trainium2 NeuronCores.

Strategy (data-parallel over N, class-slot layout):
  - Host groups rows by label and splits each class evenly across the 8
    cores. Each core receives x F-major (features on partitions) as
    xt [128, 16*4096] fp16: class c occupies the fixed column slot
    [c*4096, (c+1)*4096), zero-padded (slot capacity 4096 covers
    count_c <= 32768 globally, ~9 sigma for the uniform label fill).
  - With labels encoded purely in the layout, the per-row scale/shift
    gather disappears: scale[c]/shift[c] are per-partition [128,1]
    scalars for the whole slot.
  - Pass 1 (per slot): s1 via DVE tensor_scalar(x*1.0, accum_out) and
    s2 via x^2-with-accum, split DVE/Act so both engines keep pace with
    the input DMA stream. fp16 ops (DVE 2x perf mode), fp32 accums.
  - AllReduce the [128, 32] (s1|s2) stats across the 8 cores.
  - Stats -> scale/shift [128,16] on-chip (mirrors reference formulas).
  - Pass 2: y = x*scale_c + shift_c IN PLACE over the resident x tiles
    (DVE tensor_scalar / Act activation(Identity, scale, bias)), then
    8 big contiguous stores. HBM traffic ~17 MB in + ~17 MB out per
    core; TensorE unused. fp16 end-to-end rel_norm ~2.3e-4.

Everything is hardcoded for the problem size: x [500000,128] f32,
labels [500000] int, gamma/beta [16,128] f32.
"""
import numpy as np

N_CORES = 8
N = 500000
F = 128
C = 16
EPS = 1e-5

SLOT = 4096                  # columns per class slot (per core)
COLS = C * SLOT              # 65536 columns per core
CHUNK = 2 * SLOT             # DMA granularity: 2 slots
NCHUNK = C // 2              # 8 chunks

_CACHE = {}


def _build():
    import concourse.bacc as bacc
    import concourse.bass as bass
    from concourse import mybir
    import concourse.tile as tile

    F32 = mybir.dt.float32
    F16 = mybir.dt.float16
    AF = mybir.ActivationFunctionType
    ALU = mybir.AluOpType

    nc = bacc.Bacc("TRN2", target_bir_lowering=False, debug=False,
                   num_devices=N_CORES)
    xt = nc.dram_tensor("xt", [F, COLS], F16, kind="ExternalInput").ap()
    gt = nc.dram_tensor("gt", [F, C], F32, kind="ExternalInput").ap()
    bt = nc.dram_tensor("bt", [F, C], F32, kind="ExternalInput").ap()
    invn = nc.dram_tensor("invn", [F, 2 * C], F32, kind="ExternalInput").ap()
    y = nc.dram_tensor("y", [F, COLS], F16, kind="ExternalOutput").ap()

    with tile.TileContext(nc) as tc:
        with (
            tc.tile_pool(name="const", bufs=1) as const,
            tc.tile_pool(name="xres", bufs=NCHUNK) as xres,
            tc.tile_pool(name="tv", bufs=2) as tvp,
            tc.tile_pool(name="ta", bufs=2) as tap,
            tc.tile_pool(name="dram", bufs=1, space="DRAM") as dram,
        ):
            gt_sb = const.tile([F, C], F32)
            nc.sync.dma_start(out=gt_sb[:], in_=gt)
            bt_sb = const.tile([F, C], F32)
            nc.sync.dma_start(out=bt_sb[:], in_=bt)
            invn_sb = const.tile([F, 2 * C], F32)
            nc.sync.dma_start(out=invn_sb[:], in_=invn)
            eps_sb = const.tile([F, 1], F32)
            nc.vector.memset(eps_sb[:], EPS)

            # ============ input DMAs (all issued upfront) ============
            xg = []
            for g in range(NCHUNK):
                x_g = xres.tile([F, CHUNK], F16, tag="x")
                src = bass.AP(tensor=xt.tensor, offset=g * CHUNK,
                              ap=[[COLS, F], [1, CHUNK]])
                eng = nc.sync if g % 2 == 0 else nc.scalar
                eng.dma_start(out=x_g[:], in_=src)
                xg.append(x_g)

            # ============ PASS 1: local stats (s1 | s2) ============
            stats_sb = const.tile([F, 2 * C], F32)
            for s in range(C):
                xsl = xg[s // 2][:, (s % 2) * SLOT:(s % 2 + 1) * SLOT]
                tv = tvp.tile([F, SLOT], F16, tag="tv")
                nc.vector.tensor_scalar(out=tv[:], in0=xsl, scalar1=1.0,
                                        scalar2=None, op0=ALU.mult,
                                        op1=ALU.add,
                                        accum_out=stats_sb[:, s:s + 1])
                if s % 4 == 0:
                    tv2 = tvp.tile([F, SLOT], F16, tag="tv")
                    nc.vector.scalar_tensor_tensor(
                        out=tv2[:], in0=xsl, scalar=1.0, in1=xsl,
                        op0=ALU.mult, op1=ALU.mult,
                        accum_out=stats_sb[:, C + s:C + s + 1])
                else:
                    ta = tap.tile([F, SLOT], F16, tag="ta")
                    nc.scalar.activation(out=ta[:], in_=xsl, func=AF.Square,
                                         accum_out=stats_sb[:, C + s:C + s + 1])

            # ============ AllReduce stats ============
            cc_in = dram.tile([F, 2 * C], F32)
            cc_out = dram.tile([F, 2 * C], F32)
            nc.sync.dma_start(out=cc_in[:], in_=stats_sb[:])
            nc.gpsimd.collective_compute(
                "AllReduce",
                mybir.AluOpType.add,
                replica_groups=[list(range(N_CORES))],
                ins=[cc_in.opt()],
                outs=[cc_out.opt()],
            )
            gstats = const.tile([F, 2 * C], F32)
            nc.scalar.dma_start(out=gstats[:], in_=cc_out[:])

            # ---- stats -> scale/shift (mirrors reference formulas) ----
            me = const.tile([F, 2 * C], F32)     # mean | E[x^2]
            nc.vector.tensor_tensor(out=me[:], in0=gstats[:], in1=invn_sb[:],
                                    op=ALU.mult)
            var = const.tile([F, C], F32)
            nc.vector.tensor_tensor(out=var[:], in0=me[:, 0:C],
                                    in1=me[:, 0:C], op=ALU.mult)
            nc.vector.tensor_tensor(out=var[:], in0=me[:, C:2 * C],
                                    in1=var[:], op=ALU.subtract)
            std = const.tile([F, C], F32)
            nc.scalar.activation(out=std[:], in_=var[:], func=AF.Sqrt,
                                 bias=eps_sb[:])
            istd = const.tile([F, C], F32)
            nc.vector.reciprocal(out=istd[:], in_=std[:])
            scale = const.tile([F, C], F32)
            nc.vector.tensor_tensor(out=scale[:], in0=gt_sb[:], in1=istd[:],
                                    op=ALU.mult)
            shift = const.tile([F, C], F32)
            nc.vector.tensor_tensor(out=shift[:], in0=me[:, 0:C],
                                    in1=scale[:], op=ALU.mult)
            nc.vector.tensor_tensor(out=shift[:], in0=bt_sb[:], in1=shift[:],
                                    op=ALU.subtract)

            # ====== PASS 2: y = x*scale_c + shift_c (in place) + stores ===
            # DVE applies slots 0-9 (stores on sync queue); Act applies
            # slots 10-15 (stores on its own queue, so each store issues
            # right after its own applies in program order).
            for s in range(10):
                xsl = xg[s // 2][:, (s % 2) * SLOT:(s % 2 + 1) * SLOT]
                nc.vector.tensor_scalar(out=xsl, in0=xsl,
                                        scalar1=scale[:, s:s + 1],
                                        scalar2=shift[:, s:s + 1],
                                        op0=ALU.mult, op1=ALU.add)
                if s % 2 == 1:
                    g = s // 2
                    dst = bass.AP(tensor=y.tensor, offset=g * CHUNK,
                                  ap=[[COLS, F], [1, CHUNK]])
                    nc.sync.dma_start(out=dst, in_=xg[g][:])
            for s in range(10, C):
                xsl = xg[s // 2][:, (s % 2) * SLOT:(s % 2 + 1) * SLOT]
                nc.scalar.activation(out=xsl, in_=xsl,
                                     func=AF.Identity,
                                     bias=shift[:, s:s + 1],
                                     scale=scale[:, s:s + 1])
                if s % 2 == 1:
                    g = s // 2
                    dst = bass.AP(tensor=y.tensor, offset=g * CHUNK,
                                  ap=[[COLS, F], [1, CHUNK]])
                    nc.scalar.dma_start(out=dst, in_=xg[g][:])
    nc.finalize()
    return nc


def _get_nc():
    if "nc" not in _CACHE:
        _CACHE["nc"] = _build()
    return _CACHE["nc"]


def _numpy_fallback(x, labels, gamma, beta):
    counts = np.maximum(np.bincount(labels, minlength=C), 1).astype(np.float32)
    s1 = np.zeros((C, F), np.float32)
    s2 = np.zeros((C, F), np.float32)
    for c in range(C):
        m = labels == c
        s1[c] = x[m].sum(0)
        s2[c] = (x[m] * x[m]).sum(0)
    mean = s1 / counts[:, None]
    var = s2 / counts[:, None] - mean * mean
    istd = 1.0 / np.sqrt(var + EPS)
    scale = gamma * istd
    shift = beta - mean * scale
    return x * scale[labels] + shift[labels]


def kernel(x, labels, gamma, beta):
    from concourse.bass_utils import run_bass_kernel_spmd

    x = np.ascontiguousarray(np.asarray(x, dtype=np.float32))
    labels_np = np.asarray(labels).astype(np.int64)
    gamma = np.ascontiguousarray(np.asarray(gamma, dtype=np.float32))
    beta = np.ascontiguousarray(np.asarray(beta, dtype=np.float32))

    counts = np.bincount(labels_np, minlength=C)
    if int(counts.max()) > N_CORES * SLOT:
        return _numpy_fallback(x, labels_np, gamma, beta)

    # group rows by label; split each class evenly across cores
    order = np.argsort(labels_np, kind="stable")
    starts = np.concatenate([[0], np.cumsum(counts)])
    chunks = [np.array_split(order[starts[c]:starts[c + 1]], N_CORES)
              for c in range(C)]

    invn = (1.0 / np.maximum(counts, 1)).astype(np.float32)
    invn2 = np.concatenate([invn, invn])
    invn_b = np.ascontiguousarray(np.broadcast_to(invn2, (F, 2 * C)))
    gt = np.ascontiguousarray(gamma.T)
    bt = np.ascontiguousarray(beta.T)

    xh = x.astype(np.float16)
    in_maps = []
    for k in range(N_CORES):
        xt_k = np.zeros((F, COLS), dtype=np.float16)
        for c in range(C):
            rows = chunks[c][k]
            xt_k[:, c * SLOT:c * SLOT + len(rows)] = xh[rows].T
        in_maps.append({"xt": xt_k, "gt": gt, "bt": bt, "invn": invn_b})

    nc = _get_nc()
    res = run_bass_kernel_spmd(nc, in_maps, core_ids=list(range(N_CORES)),
                               **_CACHE.get("run_kwargs", {}))
    _CACHE["last_results"] = res

    y = np.empty((N, F), dtype=np.float32)
    for k in range(N_CORES):
        yk = res.results[k]["y"]
        for c in range(C):
            rows = chunks[c][k]
            y[rows] = yk[:, c * SLOT:c * SLOT + len(rows)].T
    return y


# revision 11
# speedup vs baseline: 1.5986x; 1.5986x over previous
"""Conditional BatchNorm1d (training mode) on 8 Trainium2 NeuronCores.

Strategy (feature-parallel, class-slot layout — no collectives):
  - Host groups rows by label into 8 row-blocks (each class split evenly
    across blocks, padded into fixed slots of 4096 columns per class).
  - Core k owns FEATURES [16k, 16k+16): its input xt [128, 16*4096] fp16
    has partition (b, f) = feature 16k+f of row-block b, columns laid out
    in the shared class-slot order. Every core sees all 500k rows, so it
    computes complete global stats for its 16 features locally — the
    cross-core AllReduce disappears entirely.
  - Pass 1 (per slot): s1 via DVE fold(hi+lo, 2x mode) + tensor_reduce;
    s2 via Act Square-activation with accum_out (one slot's s2 on DVE to
    balance). fp32 accumulation into stats[(b,f), c].
  - Block fold + broadcast in ONE PE mask-matmul: A[i,j] = (i%16==j%16);
    gstats[(b',f), c] = sum_b stats[(b,f), c].
  - Stats -> scale/shift [128,16] on-chip (mirrors reference formulas).
  - Pass 2: y = x*scale_c + shift_c IN PLACE over the resident x tiles
    (DVE tensor_scalar 4x mode / Act activation), then 8 big contiguous
    stores. ~17 MB in + ~17 MB out per core. fp16 rel_norm ~2.3e-4.

Everything is hardcoded for the problem size: x [500000,128] f32,
labels [500000] int, gamma/beta [16,128] f32.
"""
import numpy as np

N_CORES = 8
N = 500000
F = 128
C = 16
EPS = 1e-5

FPC = F // N_CORES           # 16 features per core
NBLK = N_CORES               # 8 row-blocks stacked on partitions
SLOT = 4096                  # columns per class slot
COLS = C * SLOT              # 65536 columns per core
HALF = SLOT // 2

_CACHE = {}


def _build():
    import concourse.bacc as bacc
    import concourse.bass as bass
    from concourse import mybir
    import concourse.tile as tile

    F32 = mybir.dt.float32
    F16 = mybir.dt.float16
    AF = mybir.ActivationFunctionType
    ALU = mybir.AluOpType

    nc = bacc.Bacc("TRN2", target_bir_lowering=False, debug=False,
                   num_devices=N_CORES)
    xt = nc.dram_tensor("xt", [F, COLS], F16, kind="ExternalInput").ap()
    gt = nc.dram_tensor("gt", [F, C], F32, kind="ExternalInput").ap()
    bt = nc.dram_tensor("bt", [F, C], F32, kind="ExternalInput").ap()
    invn = nc.dram_tensor("invn", [F, 2 * C], F32, kind="ExternalInput").ap()
    amask = nc.dram_tensor("amask", [F, F], F32, kind="ExternalInput").ap()
    y = nc.dram_tensor("y", [F, COLS], F16, kind="ExternalOutput").ap()

    with tile.TileContext(nc) as tc:
        with (
            tc.tile_pool(name="const", bufs=1) as const,
            tc.tile_pool(name="xres", bufs=C) as xres,
            tc.tile_pool(name="tv", bufs=2) as tvp,
            tc.tile_pool(name="ta", bufs=2) as tap,
            tc.tile_pool(name="ps", bufs=1, space="PSUM") as psp,
        ):
            gt_sb = const.tile([F, C], F32)
            nc.sync.dma_start(out=gt_sb[:], in_=gt)
            bt_sb = const.tile([F, C], F32)
            nc.sync.dma_start(out=bt_sb[:], in_=bt)
            invn_sb = const.tile([F, 2 * C], F32)
            nc.sync.dma_start(out=invn_sb[:], in_=invn)
            amask_sb = const.tile([F, F], F32)
            nc.sync.dma_start(out=amask_sb[:], in_=amask)
            eps_sb = const.tile([F, 1], F32)
            nc.vector.memset(eps_sb[:], EPS)

            # ============ input DMAs (all issued upfront) ============
            xg = []
            for s in range(C):
                x_s = xres.tile([F, SLOT], F16, tag="x")
                src = bass.AP(tensor=xt.tensor, offset=s * SLOT,
                              ap=[[COLS, F], [1, SLOT]])
                eng = nc.sync if s % 2 == 0 else nc.scalar
                eng.dma_start(out=x_s[:], in_=src)
                xg.append(x_s)

            # ============ PASS 1: per-(block,feature) stats ============
            stats_sb = const.tile([F, 2 * C], F32)
            for s in range(C):
                xsl = xg[s][:]
                # s1: fold halves (DVE 2x) then reduce 2048 cols
                tv = tvp.tile([F, HALF], F16, tag="tv")
                nc.vector.tensor_tensor(out=tv[:], in0=xg[s][:, 0:HALF],
                                        in1=xg[s][:, HALF:SLOT], op=ALU.add)
                nc.vector.tensor_reduce(out=stats_sb[:, s:s + 1], in_=tv[:],
                                        axis=mybir.AxisListType.X, op=ALU.add)
                # s2
                if s == 8:
                    tv2 = tvp.tile([F, SLOT], F16, tag="tv2")
                    nc.vector.scalar_tensor_tensor(
                        out=tv2[:], in0=xsl, scalar=1.0, in1=xsl,
                        op0=ALU.mult, op1=ALU.mult,
                        accum_out=stats_sb[:, C + s:C + s + 1])
                else:
                    ta = tap.tile([F, SLOT], F16, tag="ta")
                    nc.scalar.activation(out=ta[:], in_=xsl, func=AF.Square,
                                         accum_out=stats_sb[:, C + s:C + s + 1])

            # ====== block fold + broadcast via PE mask-matmul ======
            psum_g = psp.tile([F, 2 * C], F32)
            nc.tensor.matmul(out=psum_g[:], lhsT=amask_sb[:], rhs=stats_sb[:],
                             start=True, stop=True)
            gstats = const.tile([F, 2 * C], F32)
            nc.vector.tensor_copy(out=gstats[:], in_=psum_g[:])

            # ---- stats -> scale/shift (mirrors reference formulas) ----
            me = const.tile([F, 2 * C], F32)     # mean | E[x^2]
            nc.vector.tensor_tensor(out=me[:], in0=gstats[:], in1=invn_sb[:],
                                    op=ALU.mult)
            var = const.tile([F, C], F32)
            nc.vector.tensor_tensor(out=var[:], in0=me[:, 0:C],
                                    in1=me[:, 0:C], op=ALU.mult)
            nc.vector.tensor_tensor(out=var[:], in0=me[:, C:2 * C],
                                    in1=var[:], op=ALU.subtract)
            std = const.tile([F, C], F32)
            nc.scalar.activation(out=std[:], in_=var[:], func=AF.Sqrt,
                                 bias=eps_sb[:])
            istd = const.tile([F, C], F32)
            nc.vector.reciprocal(out=istd[:], in_=std[:])
            scale = const.tile([F, C], F32)
            nc.vector.tensor_tensor(out=scale[:], in0=gt_sb[:], in1=istd[:],
                                    op=ALU.mult)
            shift = const.tile([F, C], F32)
            nc.vector.tensor_tensor(out=shift[:], in0=me[:, 0:C],
                                    in1=scale[:], op=ALU.mult)
            nc.vector.tensor_tensor(out=shift[:], in0=bt_sb[:], in1=shift[:],
                                    op=ALU.subtract)

            # ====== PASS 2: y = x*scale_c + shift_c (in place) + stores ===
            for s in range(12):
                xsl = xg[s][:]
                nc.vector.tensor_scalar(out=xsl, in0=xsl,
                                        scalar1=scale[:, s:s + 1],
                                        scalar2=shift[:, s:s + 1],
                                        op0=ALU.mult, op1=ALU.add)
                dst = bass.AP(tensor=y.tensor, offset=s * SLOT,
                              ap=[[COLS, F], [1, SLOT]])
                nc.sync.dma_start(out=dst, in_=xg[s][:])
            for s in range(12, C):
                xsl = xg[s][:]
                nc.scalar.activation(out=xsl, in_=xsl,
                                     func=AF.Identity,
                                     bias=shift[:, s:s + 1],
                                     scale=scale[:, s:s + 1])
                dst = bass.AP(tensor=y.tensor, offset=s * SLOT,
                              ap=[[COLS, F], [1, SLOT]])
                nc.scalar.dma_start(out=dst, in_=xg[s][:])
    nc.finalize()
    return nc


def _get_nc():
    if "nc" not in _CACHE:
        _CACHE["nc"] = _build()
    return _CACHE["nc"]


def _numpy_fallback(x, labels, gamma, beta):
    counts = np.maximum(np.bincount(labels, minlength=C), 1).astype(np.float32)
    s1 = np.zeros((C, F), np.float32)
    s2 = np.zeros((C, F), np.float32)
    for c in range(C):
        m = labels == c
        s1[c] = x[m].sum(0)
        s2[c] = (x[m] * x[m]).sum(0)
    mean = s1 / counts[:, None]
    var = s2 / counts[:, None] - mean * mean
    istd = 1.0 / np.sqrt(var + EPS)
    scale = gamma * istd
    shift = beta - mean * scale
    return x * scale[labels] + shift[labels]


def kernel(x, labels, gamma, beta):
    from concourse.bass_utils import run_bass_kernel_spmd

    x = np.ascontiguousarray(np.asarray(x, dtype=np.float32))
    labels_np = np.asarray(labels).astype(np.int64)
    gamma = np.ascontiguousarray(np.asarray(gamma, dtype=np.float32))
    beta = np.ascontiguousarray(np.asarray(beta, dtype=np.float32))

    counts = np.bincount(labels_np, minlength=C)
    if int(counts.max()) > NBLK * SLOT:
        return _numpy_fallback(x, labels_np, gamma, beta)

    # group rows by label; split each class evenly across the 8 row-blocks
    order = np.argsort(labels_np, kind="stable")
    starts = np.concatenate([[0], np.cumsum(counts)])
    chunks = [np.array_split(order[starts[c]:starts[c + 1]], NBLK)
              for c in range(C)]

    invn = (1.0 / np.maximum(counts, 1)).astype(np.float32)
    invn2 = np.concatenate([invn, invn])
    invn_b = np.ascontiguousarray(np.broadcast_to(invn2, (F, 2 * C)))
    amask = np.tile(np.eye(FPC, dtype=np.float32), (NBLK, NBLK))
    amask = np.ascontiguousarray(amask)

    # build the 8 row-block matrices [128 features, COLS] once, then
    # redistribute: core k takes feature rows [16k,16k+16) of every block.
    xh = x.astype(np.float16)
    blocks = []
    for b in range(NBLK):
        xb = np.zeros((F, COLS), dtype=np.float16)
        for c in range(C):
            rows = chunks[c][b]
            xb[:, c * SLOT:c * SLOT + len(rows)] = xh[rows].T
        blocks.append(xb)

    in_maps = []
    for k in range(N_CORES):
        fsl = slice(k * FPC, (k + 1) * FPC)
        xt_k = np.concatenate([blocks[b][fsl] for b in range(NBLK)], axis=0)
        gt_k = np.ascontiguousarray(
            np.tile(gamma.T[fsl], (NBLK, 1)))          # [(b,f), c]
        bt_k = np.ascontiguousarray(np.tile(beta.T[fsl], (NBLK, 1)))
        in_maps.append({"xt": np.ascontiguousarray(xt_k), "gt": gt_k,
                        "bt": bt_k, "invn": invn_b, "amask": amask})

    nc = _get_nc()
    res = run_bass_kernel_spmd(nc, in_maps, core_ids=list(range(N_CORES)),
                               **_CACHE.get("run_kwargs", {}))
    _CACHE["last_results"] = res

    y = np.empty((N, F), dtype=np.float32)
    for k in range(N_CORES):
        yk = res.results[k]["y"]
        fsl = slice(k * FPC, (k + 1) * FPC)
        for b in range(NBLK):
            ybf = yk[b * FPC:(b + 1) * FPC]
            for c in range(C):
                rows = chunks[c][b]
                y[rows, fsl] = ybf[:, c * SLOT:c * SLOT + len(rows)].T
    return y


# revision 13
# speedup vs baseline: 1.8540x; 1.1598x over previous
"""Conditional BatchNorm1d (training mode) on 8 Trainium2 NeuronCores.

Strategy (feature-parallel, class-slot layout — no collectives):
  - Host groups rows by label into 8 row-blocks (each class split evenly
    across blocks, padded into fixed slots of 4096 columns per class).
  - Core k owns FEATURES [16k, 16k+16): its input xt [128, 16*4096] fp16
    has partition (b, f) = feature 16k+f of row-block b, columns laid out
    in the shared class-slot order. Every core sees all 500k rows, so it
    computes complete global stats for its 16 features locally — the
    cross-core AllReduce disappears entirely.
  - Pass 1 (per slot): s1 via DVE fold(hi+lo, 2x mode) + tensor_reduce;
    s2 via Act Square-activation with accum_out (one slot's s2 on DVE to
    balance). fp32 accumulation into stats[(b,f), c].
  - Block fold + broadcast in ONE PE mask-matmul: A[i,j] = (i%16==j%16);
    gstats[(b',f), c] = sum_b stats[(b,f), c].
  - Stats -> scale/shift [128,16] on-chip (mirrors reference formulas).
  - Pass 2: y = x*scale_c + shift_c IN PLACE over the resident x tiles
    (DVE tensor_scalar 4x mode / Act activation), then 8 big contiguous
    stores. ~17 MB in + ~17 MB out per core. fp16 rel_norm ~2.3e-4.

Everything is hardcoded for the problem size: x [500000,128] f32,
labels [500000] int, gamma/beta [16,128] f32.
"""
import numpy as np

N_CORES = 8
N = 500000
F = 128
C = 16
EPS = 1e-5

FPC = F // N_CORES           # 16 features per core
NBLK = N_CORES               # 8 row-blocks stacked on partitions
SLOT = 4096                  # columns per class slot
COLS = C * SLOT              # 65536 columns per core
HALF = SLOT // 2

_CACHE = {}


def _build():
    import concourse.bacc as bacc
    import concourse.bass as bass
    from concourse import mybir
    import concourse.tile as tile

    F32 = mybir.dt.float32
    F16 = mybir.dt.float16
    AF = mybir.ActivationFunctionType
    ALU = mybir.AluOpType

    nc = bacc.Bacc("TRN2", target_bir_lowering=False, debug=False,
                   num_devices=N_CORES)
    xt = nc.dram_tensor("xt", [F, COLS], F16, kind="ExternalInput").ap()
    gt = nc.dram_tensor("gt", [F, C], F32, kind="ExternalInput").ap()
    bt = nc.dram_tensor("bt", [F, C], F32, kind="ExternalInput").ap()
    invn = nc.dram_tensor("invn", [F, 2 * C], F32, kind="ExternalInput").ap()
    amask = nc.dram_tensor("amask", [F, F], F32, kind="ExternalInput").ap()
    y = nc.dram_tensor("y", [F, COLS], F16, kind="ExternalOutput").ap()

    with tile.TileContext(nc) as tc:
        with (
            tc.tile_pool(name="const", bufs=1) as const,
            tc.tile_pool(name="xs8", bufs=8) as xs8,
            tc.tile_pool(name="xch", bufs=4) as xch,
            tc.tile_pool(name="tv", bufs=2) as tvp,
            tc.tile_pool(name="tq", bufs=2) as tqp,
            tc.tile_pool(name="ta", bufs=2) as tap,
            tc.tile_pool(name="ps", bufs=1, space="PSUM") as psp,
        ):
            # ====== input DMAs first (x before consts; Act queue gets only
            # 4 fresh-semaphore issues so its stream never stalls) ======
            xg = []
            for s in range(8):
                x_s = xs8.tile([F, SLOT], F16, tag="x")
                src = bass.AP(tensor=xt.tensor, offset=s * SLOT,
                              ap=[[COLS, F], [1, SLOT]])
                eng = nc.sync if s % 2 == 0 else nc.scalar
                eng.dma_start(out=x_s[:], in_=src)
                xg.append(x_s)
            for g in range(4):
                x_c = xch.tile([F, 2 * SLOT], F16, tag="xc")
                src = bass.AP(tensor=xt.tensor, offset=(8 + 2 * g) * SLOT,
                              ap=[[COLS, F], [1, 2 * SLOT]])
                nc.sync.dma_start(out=x_c[:], in_=src)
                xg.append(x_c)

            def xsl(s):
                if s < 8:
                    return xg[s][:]
                t = xg[8 + (s - 8) // 2]
                return t[:, (s % 2) * SLOT:(s % 2 + 1) * SLOT]

            gt_sb = const.tile([F, C], F32)
            nc.sync.dma_start(out=gt_sb[:], in_=gt)
            bt_sb = const.tile([F, C], F32)
            nc.sync.dma_start(out=bt_sb[:], in_=bt)
            invn_sb = const.tile([F, 2 * C], F32)
            nc.sync.dma_start(out=invn_sb[:], in_=invn)
            amask_sb = const.tile([F, F], F32)
            nc.sync.dma_start(out=amask_sb[:], in_=amask)
            eps_sb = const.tile([F, 1], F32)
            nc.vector.memset(eps_sb[:], EPS)

            # ============ PASS 1: per-(block,feature) stats ============
            stats_sb = const.tile([F, 2 * C], F32)
            for s in range(C):
                x_s = xsl(s)
                # s1: two fold levels (DVE 2x mode) then reduce 1024 cols
                tv = tvp.tile([F, HALF], F16, tag="tv")
                nc.vector.tensor_tensor(out=tv[:], in0=x_s[:, 0:HALF],
                                        in1=x_s[:, HALF:SLOT], op=ALU.add)
                tq = tqp.tile([F, HALF // 2], F16, tag="tq")
                nc.vector.tensor_tensor(out=tq[:], in0=tv[:, 0:HALF // 2],
                                        in1=tv[:, HALF // 2:HALF], op=ALU.add)
                nc.vector.tensor_reduce(out=stats_sb[:, s:s + 1], in_=tq[:],
                                        axis=mybir.AxisListType.X, op=ALU.add)
                # s2
                if s >= 14:
                    tv2 = tvp.tile([F, SLOT], F16, tag="tv2")
                    nc.vector.scalar_tensor_tensor(
                        out=tv2[:], in0=x_s, scalar=1.0, in1=x_s,
                        op0=ALU.mult, op1=ALU.mult,
                        accum_out=stats_sb[:, C + s:C + s + 1])
                else:
                    ta = tap.tile([F, SLOT], F16, tag="ta")
                    nc.scalar.activation(out=ta[:], in_=x_s, func=AF.Square,
                                         accum_out=stats_sb[:, C + s:C + s + 1])

            # ====== block fold + broadcast via PE mask-matmul ======
            psum_g = psp.tile([F, 2 * C], F32)
            nc.tensor.matmul(out=psum_g[:], lhsT=amask_sb[:], rhs=stats_sb[:],
                             start=True, stop=True)
            gstats = const.tile([F, 2 * C], F32)
            nc.vector.tensor_copy(out=gstats[:], in_=psum_g[:])

            # ---- stats -> scale/shift (mirrors reference formulas) ----
            me = const.tile([F, 2 * C], F32)     # mean | E[x^2]
            nc.vector.tensor_tensor(out=me[:], in0=gstats[:], in1=invn_sb[:],
                                    op=ALU.mult)
            var = const.tile([F, C], F32)
            nc.vector.tensor_tensor(out=var[:], in0=me[:, 0:C],
                                    in1=me[:, 0:C], op=ALU.mult)
            nc.vector.tensor_tensor(out=var[:], in0=me[:, C:2 * C],
                                    in1=var[:], op=ALU.subtract)
            std = const.tile([F, C], F32)
            nc.scalar.activation(out=std[:], in_=var[:], func=AF.Sqrt,
                                 bias=eps_sb[:])
            istd = const.tile([F, C], F32)
            nc.vector.reciprocal(out=istd[:], in_=std[:])
            scale = const.tile([F, C], F32)
            nc.vector.tensor_tensor(out=scale[:], in0=gt_sb[:], in1=istd[:],
                                    op=ALU.mult)
            shift = const.tile([F, C], F32)
            nc.vector.tensor_tensor(out=shift[:], in0=me[:, 0:C],
                                    in1=scale[:], op=ALU.mult)
            nc.vector.tensor_tensor(out=shift[:], in0=bt_sb[:], in1=shift[:],
                                    op=ALU.subtract)

            # ====== PASS 2: y = x*scale_c + shift_c (in place) + stores ===
            for s in range(12):
                x_s = xsl(s)
                nc.vector.tensor_scalar(out=x_s, in0=x_s,
                                        scalar1=scale[:, s:s + 1],
                                        scalar2=shift[:, s:s + 1],
                                        op0=ALU.mult, op1=ALU.add)
                dst = bass.AP(tensor=y.tensor, offset=s * SLOT,
                              ap=[[COLS, F], [1, SLOT]])
                nc.sync.dma_start(out=dst, in_=x_s)
            for s in range(12, C):
                x_s = xsl(s)
                nc.scalar.activation(out=x_s, in_=x_s,
                                     func=AF.Identity,
                                     bias=shift[:, s:s + 1],
                                     scale=scale[:, s:s + 1])
                dst = bass.AP(tensor=y.tensor, offset=s * SLOT,
                              ap=[[COLS, F], [1, SLOT]])
                nc.scalar.dma_start(out=dst, in_=x_s)
    nc.finalize()
    return nc


def _get_nc():
    if "nc" not in _CACHE:
        _CACHE["nc"] = _build()
    return _CACHE["nc"]


def _numpy_fallback(x, labels, gamma, beta):
    counts = np.maximum(np.bincount(labels, minlength=C), 1).astype(np.float32)
    s1 = np.zeros((C, F), np.float32)
    s2 = np.zeros((C, F), np.float32)
    for c in range(C):
        m = labels == c
        s1[c] = x[m].sum(0)
        s2[c] = (x[m] * x[m]).sum(0)
    mean = s1 / counts[:, None]
    var = s2 / counts[:, None] - mean * mean
    istd = 1.0 / np.sqrt(var + EPS)
    scale = gamma * istd
    shift = beta - mean * scale
    return x * scale[labels] + shift[labels]


def kernel(x, labels, gamma, beta):
    from concourse.bass_utils import run_bass_kernel_spmd

    x = np.ascontiguousarray(np.asarray(x, dtype=np.float32))
    labels_np = np.asarray(labels).astype(np.int64)
    gamma = np.ascontiguousarray(np.asarray(gamma, dtype=np.float32))
    beta = np.ascontiguousarray(np.asarray(beta, dtype=np.float32))

    counts = np.bincount(labels_np, minlength=C)
    if int(counts.max()) > NBLK * SLOT:
        return _numpy_fallback(x, labels_np, gamma, beta)

    # group rows by label; split each class evenly across the 8 row-blocks
    order = np.argsort(labels_np, kind="stable")
    starts = np.concatenate([[0], np.cumsum(counts)])
    chunks = [np.array_split(order[starts[c]:starts[c + 1]], NBLK)
              for c in range(C)]

    invn = (1.0 / np.maximum(counts, 1)).astype(np.float32)
    invn2 = np.concatenate([invn, invn])
    invn_b = np.ascontiguousarray(np.broadcast_to(invn2, (F, 2 * C)))
    amask = np.tile(np.eye(FPC, dtype=np.float32), (NBLK, NBLK))
    amask = np.ascontiguousarray(amask)

    # build the 8 row-block matrices [128 features, COLS] once, then
    # redistribute: core k takes feature rows [16k,16k+16) of every block.
    xh = x.astype(np.float16)
    blocks = []
    for b in range(NBLK):
        xb = np.zeros((F, COLS), dtype=np.float16)
        for c in range(C):
            rows = chunks[c][b]
            xb[:, c * SLOT:c * SLOT + len(rows)] = xh[rows].T
        blocks.append(xb)

    in_maps = []
    for k in range(N_CORES):
        fsl = slice(k * FPC, (k + 1) * FPC)
        xt_k = np.concatenate([blocks[b][fsl] for b in range(NBLK)], axis=0)
        gt_k = np.ascontiguousarray(
            np.tile(gamma.T[fsl], (NBLK, 1)))          # [(b,f), c]
        bt_k = np.ascontiguousarray(np.tile(beta.T[fsl], (NBLK, 1)))
        in_maps.append({"xt": np.ascontiguousarray(xt_k), "gt": gt_k,
                        "bt": bt_k, "invn": invn_b, "amask": amask})

    nc = _get_nc()
    res = run_bass_kernel_spmd(nc, in_maps, core_ids=list(range(N_CORES)),
                               **_CACHE.get("run_kwargs", {}))
    _CACHE["last_results"] = res

    y = np.empty((N, F), dtype=np.float32)
    for k in range(N_CORES):
        yk = res.results[k]["y"]
        fsl = slice(k * FPC, (k + 1) * FPC)
        for b in range(NBLK):
            ybf = yk[b * FPC:(b + 1) * FPC]
            for c in range(C):
                rows = chunks[c][b]
                y[rows, fsl] = ybf[:, c * SLOT:c * SLOT + len(rows)].T
    return y
